# revision 37
# baseline (speedup 1.0000x reference)
"""AtomGIN (3-layer GIN message passing) on 8 Trainium2 NeuronCores.

Strategy (graph/data parallel, dst-partitioned):
  - Nodes split evenly across 8 cores (core c owns rows [c*6250, (c+1)*6250)).
  - Layer 0 is fully analytic: h0 is a lookup into the 9-row atom table, so
    aggr0[:, v] = t9.T @ cnt9[:, v] + t15_0.T @ cnt[:, v] with host-built
    per-destination count matrices (self-loop terms folded into the counts).
    No embedding materialization, no initial AllGather, no layer-0 gathers.
  - Layers 1-2: each core aggregates the edges whose DESTINATION is local via
    PSUM matmul accumulation per 128-dst window: msg_tile.T @ onehot. Edges
    are packed DENSELY into gather tiles (sorted by dst); a window's matmuls
    cover the tile range [ta, tb) that holds its edges on ANY core, with
    per-(window, tile) one-hot matrices precomputed host-side (zero rows for
    out-of-window edges), so no gather index is wasted on padding.
    Self-loop contribution = transpose of the local h window (one matmul).
    Edge-embedding term stays analytic (t15/cnt).
  - h is replicated in every core's HBM via AllGather after each layer;
    per-edge source rows are fetched with GPSIMD dma_gather (int16 indices;
    h addressed as two halves so indices fit 15 bits). Gathers round-robin
    over 4 SWDGE queues; 15-tile calls keep each call's 121 descriptors
    inside the 128-entry SWDGE ring (desc-gen is the kernel bottleneck).
  - The MLP runs feature-major, batched 512 columns wide; BN statistics are
    accumulated per batch (hidden under the gathers) + a tiny AllReduce; the
    normalize runs node-major (after early per-window raw transposes) in
    window chunks pipelined with the h store.
"""

import numpy as np

N = 50000
E = 500000
D = 128
L = 3
BN_EPS = 1e-5
P = 128
NCORES = 8
NPC = N // NCORES            # nodes per core
NT = (NPC + P - 1) // P      # node windows per core
NPAD = NT * P
HHALF = NPAD // 2            # (unused) half-shard row count
CH_TILES = 15                # edge tiles per dma_gather call; 15*128/16+1 = 121
                             # descriptors fits the 128-entry SWDGE ring
NQ = 4                       # SWDGE queues to round-robin
MB = 4                       # node windows per MLP batch (512 cols)
NCH = 12                     # node windows per normalize/store chunk


def _wrap_idx_cols(idx2d):
    """[rows] int -> dma_gather wrapped layout [128, rows//16] int16.

    Index i lives at (partition i%16, col i//16), replicated 8x down the
    partition axis (one copy per Q7 core).
    """
    n = idx2d.shape[0]
    w = idx2d.reshape(n // 16, 16).T.astype(np.int16)
    return np.tile(w, (8, 1))


def _preprocess(x, edge_index, edge_attr):
    """Host-side integer preprocessing. Returns (schedule, per-core arrays)."""
    x = np.asarray(x)
    ei = np.asarray(edge_index)
    ea = np.asarray(edge_attr)

    code_a = (x[:, 0] * 3 + x[:, 1]).astype(np.int64)          # [N] in 0..8
    src = ei[0].astype(np.int64)
    dst = ei[1].astype(np.int64)
    ecode = (ea[:, 0] * 3 + ea[:, 1]).astype(np.int64)          # [E] in 0..8

    core = dst // NPC
    dst_local = dst - core * NPC
    # permuted h row for a source node: shards are stored partition-major
    # ([p, t, d] flat), so node (c, i) lands at row c*NPAD + (i%P)*NT + i//P
    s_core = src // NPC
    s_loc = src - s_core * NPC
    perm = (s_loc % P) * NT + s_loc // P
    is_own = s_core == core

    grow = s_core * NPAD + perm

    # --- dense per-stream schedule: edges sorted by dst window, packed into
    # tiles with no padding; window nt uses tiles [ta[nt], tb[nt]) (the union
    # of per-core ranges) with per-(window, tile) one-hots. The "ow" stream
    # (source owned by the destination core) gathers from h_shard, so its
    # descriptor generation overlaps the AllGather; remote edges split by row
    # PARITY (strided gather base) so indices fit int16 and the two streams
    # stay balanced across cores. ---
    sched = {}
    arrays = {}
    for sname in ("ow", "ev", "od"):
        if sname == "ow":
            sel = is_own
            sidx = perm
        elif sname == "ev":
            sel = ~is_own & (grow % 2 == 0)
            sidx = grow // 2
        else:
            sel = ~is_own & (grow % 2 == 1)
            sidx = grow // 2
        n_c = np.zeros(NCORES, np.int64)
        f = np.zeros((NCORES, NT + 1), np.int64)
        per_core = []
        for c in range(NCORES):
            m = sel & (core == c)
            s_s = sidx[m]
            dl = dst_local[m]
            order = np.argsort(dl, kind="stable")
            s_s, dl = s_s[order], dl[order]
            n_c[c] = len(dl)
            f[c, 1:] = np.cumsum(np.bincount(dl // P, minlength=NT))
            per_core.append((s_s, dl))
        ET = int(np.ceil(n_c / P).max())
        ta = (f[:, :NT] // P).min(axis=0)
        tb = np.maximum(np.ceil(f[:, 1:] / P).astype(np.int64).max(axis=0), ta)
        width = tb - ta
        ohpos = np.zeros(NT + 1, np.int64)
        ohpos[1:] = np.cumsum(width)
        TOT = int(ohpos[-1])
        gidx = np.zeros((NCORES, ET * P), np.int64)
        OH = np.zeros((NCORES, TOT, P, P), np.uint8)
        for c in range(NCORES):
            s_s, dl = per_core[c]
            slots = np.arange(len(dl))
            gidx[c, : len(dl)] = s_s
            w = dl // P
            t = slots // P
            OH[c, ohpos[w] + t - ta[w], slots % P, dl - w * P] = 1
        sched[sname] = dict(ET=ET, ta=ta.tolist(), tb=tb.tolist(),
                            ohpos=ohpos.tolist(), TOT=TOT)
        arrays["gidx_" + sname] = gidx
        arrays["oh_" + sname] = OH

    # edge-code count matrix [cores, 16, NPAD]: real edges + self-loop code 12
    cnt = np.zeros((NCORES, 16, NPAD), np.float32)
    np.add.at(cnt, (core, ecode, dst_local), 1.0)
    allc = np.arange(N, dtype=np.int64)
    cnt[allc // NPC, 12, allc - (allc // NPC) * NPC] += 1.0

    # atom-code count matrix [cores, 16, NPAD]: src codes of real edges
    # + each node's own code (self-loop h term)
    cnt9 = np.zeros((NCORES, 16, NPAD), np.float32)
    np.add.at(cnt9, (core, code_a[src], dst_local), 1.0)
    np.add.at(cnt9, (allc // NPC, code_a, allc - (allc // NPC) * NPC), 1.0)

    arrays["cnt"] = cnt
    arrays["cnt32"] = np.concatenate([cnt9, cnt], axis=1)  # [cores, 32, NPAD]
    return sched, arrays


def _build(sched):
    """Build the SPMD Bacc graph (one graph, run on all 8 cores)."""
    import concourse.bacc as bacc
    import concourse.bass as bass
    import concourse.mybir as mybir
    from concourse.tile import TileContext

    f32 = mybir.dt.float32
    bf16 = mybir.dt.bfloat16
    i16 = mybir.dt.int16
    ACT = mybir.ActivationFunctionType
    ALU = mybir.AluOpType

    nc = bacc.Bacc("TRN2", target_bir_lowering=False, debug=False,
                   num_devices=NCORES, num_swdge_queues=NQ)

    def inp(name, shape, dt):
        return nc.declare_dram_parameter(name, list(shape), dt, isOutput=False)

    STREAMS = ("ow", "ev", "od")
    ET = {s: sched[s]["ET"] for s in STREAMS}
    TOT = {s: sched[s]["TOT"] for s in STREAMS}
    gidx_in = {s: inp(f"gidx_{s}", [P, ET[s] * 8], i16) for s in STREAMS}
    f8 = mybir.dt.float8e4
    oh_in = {s: inp(f"oh_{s}", [P, TOT[s] * P], f8) for s in STREAMS}
    cnt_in = inp("cnt", [16, NPAD], bf16)
    cnt32_in = inp("cnt32", [32, NPAD], bf16)
    w1e_in = inp("w1e", [32, 2 * D], bf16)
    prm_in = inp("prmT", [P, 8 * L], f32)      # cols: l*8 + (b1a,b1b,b2,gamma,beta,0,0,0)
    w1_in = inp("w1", [L, D, 2 * D], bf16)
    w2_in = inp("w2", [L, 2 * D, D], bf16)
    t15_in = inp("t15", [L, 16, D], bf16)
    idf_in = inp("identf", [P, P], f32)
    idb_in = inp("identb", [P, P], f8)
    out_ext = nc.declare_dram_parameter("out", [NPC, D], f32, isOutput=True)

    NROWS = NCORES * NPAD
    h_loc = nc.dram_tensor("h_loc", [NROWS, D], bf16)
    h_shard = nc.dram_tensor("h_shard", [NPAD, D], bf16)
    st_loc = nc.dram_tensor("st_loc", [P, 2], f32)
    st_glob = nc.dram_tensor("st_glob", [P, 2], f32, addr_space="Shared")
    RG = [list(range(NCORES))]

    nfull = NPC // P
    rem = NPC - nfull * P
    inv_n = 1.0 / float(N)
    NBATCH = (NT + MB - 1) // MB

    with TileContext(nc) as tc:
        with tc.tile_pool(name="cst", bufs=1) as cp, \
             tc.tile_pool(name="big", bufs=1) as bp, \
             tc.tile_pool(name="wgt", bufs=2) as wp, \
             tc.tile_pool(name="msg", bufs=8) as mp, \
             tc.tile_pool(name="ohp", bufs=8) as op_, \
             tc.tile_pool(name="own", bufs=5) as owp, \
             tc.tile_pool(name="act", bufs=4) as ap_, \
             tc.tile_pool(name="sml", bufs=1) as sp:

            # ---- persistent constants / inputs in SBUF ----
            def load(pool, shape, dt, src, tag):
                t = pool.tile(list(shape), dt, tag=tag)
                nc.sync.dma_start(out=t[:], in_=src)
                return t

            # layer-0 operands first so the batch loop starts ASAP;
            # gather indices (only needed for layer 1) load behind them
            cnt32_sb = load(cp, [32, NPAD], bf16, cnt32_in[:, :], "cnt32")
            w1e_sb = load(cp, [32, 2 * D], bf16, w1e_in[:, :], "w1e")
            cnt_sb = load(cp, [16, NPAD], bf16, cnt_in[:, :], "cnt")
            prm_sb = load(cp, [P, 8 * L], f32, prm_in[:, :], "prm")
            idf_sb = load(cp, [P, P], f32, idf_in[:, :], "idf")
            gi = {s: load(cp, [P, ET[s] * 8], i16, gidx_in[s][:, :], f"gi_{s}")
                  for s in STREAMS}
            idb_sb = load(cp, [P, P], f8, idb_in[:, :], "idb")

            ones1 = sp.tile([1, P], f32)
            nc.vector.memset(ones1[:], 1.0)

            # ---- big working buffers ----
            out_raw = bp.tile([P, NT * D], f32)     # raw post-MLP, node-major
            nrm = bp.tile([P, NT * D], f32)         # normalize scratch
            h_nm = bp.tile([P, NT * D], bf16)       # node-major h (current)

            # small stat tiles
            stats_sb = sp.tile([P, 2], f32)
            gstats_sb = sp.tile([P, 2], f32)
            stats_p1 = sp.tile([P, 16], f32)
            stats_p2 = sp.tile([P, 16], f32)
            mean_c = sp.tile([P, 1], f32)
            ex2_c = sp.tile([P, 1], f32)
            msq_c = sp.tile([P, 1], f32)
            var_c = sp.tile([P, 1], f32)
            sd_c = sp.tile([P, 1], f32)
            rstd_c = sp.tile([P, 1], f32)
            kc2 = sp.tile([P, 2], f32)
            tmp_c = sp.tile([P, 1], f32)
            krow = sp.tile([1, P], f32)
            crow = sp.tile([1, P], f32)
            kb_sb = sp.tile([P, P], f32)
            cb_sb = sp.tile([P, P], f32)

            own_chunks = {}   # layer -> {ci: msg tile} gathered from h_shard

            # ================= layers =================
            for l in range(L):
                w1_sb = load(wp, [D, 2 * D], bf16, w1_in[l, :, :], tag="w1")
                w2a_sb = load(wp, [D, D], bf16, w2_in[l, 0:D, :], tag="w2a")
                w2b_sb = load(wp, [D, D], bf16, w2_in[l, D:2 * D, :], tag="w2b")
                t15_sb = load(wp, [16, D], bf16, t15_in[l, :, :], tag="t15")

                b1a = prm_sb[:, l * 8 + 0:l * 8 + 1]
                b1b = prm_sb[:, l * 8 + 1:l * 8 + 2]
                b2 = prm_sb[:, l * 8 + 2:l * 8 + 3]
                gam = prm_sb[:, l * 8 + 3:l * 8 + 4]
                bet = prm_sb[:, l * 8 + 4:l * 8 + 5]

                # lazy chunked loads: msg chunks keyed by gather-tile block,
                # one-hot chunks keyed by flat (window, tile) position block.
                # "ow" msg chunks were eagerly gathered from h_shard right
                # after the previous layer's AllGather trigger. Chunks taper
                # at both ends: a small first chunk fills the pipeline sooner
                # after the AllGather, small last chunks cut the tail payload
                # drain (per-queue drain is ~200ns/descriptor, serial).
                mchunks = {"ow": own_chunks.get(l, {}), "ev": {}, "od": {}}
                ochunks = {"ow": {}, "ev": {}, "od": {}}
                hl = h_loc.ap()

                def chunk_table(et):
                    sizes = []
                    rem = et
                    sizes.append(min(8, rem)); rem -= sizes[-1]
                    while rem > 22:
                        sizes.append(CH_TILES); rem -= CH_TILES
                    if rem > 8:
                        sizes.append(rem - 7); sizes.append(7)
                    elif rem > 0:
                        sizes.append(rem)
                    starts, t2c, c0 = [], [], 0
                    for ci, sz in enumerate(sizes):
                        starts.append(c0); t2c += [ci] * sz; c0 += sz
                    return sizes, starts, t2c

                ctab = {s2: chunk_table(ET[s2]) for s2 in ("ev", "od")}

                def msg_slice(s, t):
                    if s == "ow":
                        ci = t // CH_TILES
                        return mchunks[s][ci][:, (t % CH_TILES) * D:
                                              (t % CH_TILES + 1) * D]
                    sizes, starts, t2c = ctab[s]
                    ci = t2c[t]
                    if ci not in mchunks[s]:
                        c0, ntile = starts[ci], sizes[ci]
                        nidx = ntile * P
                        g = mp.tile([P, CH_TILES * D], bf16, tag="msg")
                        nc.gpsimd.dma_gather(
                            out_ap=g[:, 0:ntile * D].rearrange("p (t e) -> p t e", e=D),
                            in_ap=bass.AP(hl.tensor, 0 if s == "ev" else D,
                                          [[2 * D, NROWS // 2], [1, D]]),
                            idxs_ap=gi[s][:, c0 * 8: c0 * 8 + nidx // 16],
                            num_idxs=nidx, num_idxs_reg=nidx, elem_size=D,
                            elem_step=2 * D,
                            single_packet=False, queue_num=0)
                        mchunks[s][ci] = g
                    j = t - ctab[s][1][ci]
                    return mchunks[s][ci][:, j * D:(j + 1) * D]

                def oh_slice(s, pos):
                    cj = pos // CH_TILES
                    if cj not in ochunks[s]:
                        ntile = min(CH_TILES, TOT[s] - cj * CH_TILES)
                        ohc = op_.tile([P, CH_TILES * P], f8, tag="ohc")
                        nc.sync.dma_start(
                            out=ohc[:, 0:ntile * P],
                            in_=oh_in[s][:, cj * CH_TILES * P:(cj * CH_TILES + ntile) * P])
                        ochunks[s][cj] = ohc
                    j = pos % CH_TILES
                    return ochunks[s][cj][:, j * P:(j + 1) * P]

                with tc.tile_pool(name="psa", bufs=2, space="PSUM") as pa, \
                     tc.tile_pool(name="psh", bufs=2, space="PSUM") as ph, \
                     tc.tile_pool(name="pso", bufs=2, space="PSUM") as po, \
                     tc.tile_pool(name="pst", bufs=2, space="PSUM") as pt:
                    for b in range(NBATCH):
                        w0 = b * MB
                        w1n = min(MB, NT - w0)
                        # ---- aggregation for the batch's windows ----
                        aggr_b = ap_.tile([P, MB * P], bf16, tag="aggr")
                        if l == 0:
                            pass   # layer 0: MLP consumes cnt32 directly
                        else:
                            for nt in range(w0, w0 + w1n):
                                psa = pa.tile([P, P], f32, tag="psa")
                                nc.tensor.matmul(
                                    out=psa[:], lhsT=h_nm[:, nt * D:(nt + 1) * D],
                                    rhs=idb_sb[:], start=True, stop=False)
                                for s in STREAMS:
                                    st = sched[s]
                                    for j in range(st["tb"][nt] - st["ta"][nt]):
                                        m = msg_slice(s, st["ta"][nt] + j)
                                        oh = oh_slice(s, st["ohpos"][nt] + j)
                                        nc.tensor.matmul(out=psa[:], lhsT=m, rhs=oh,
                                                         start=False, stop=False)
                                nc.tensor.matmul(out=psa[:], lhsT=t15_sb[:],
                                                 rhs=cnt_sb[:, nt * P:(nt + 1) * P],
                                                 start=False, stop=True)
                                nc.scalar.activation(
                                    aggr_b[:, (nt - w0) * P:(nt - w0 + 1) * P],
                                    psa[:], ACT.Copy)

                        # ---- batched MLP (512 cols) ----
                        cols = w1n * P
                        c0 = w0 * P
                        if l == 0:
                            lhs1, lhs2 = w1e_sb[:, 0:D], w1e_sb[:, D:2 * D]
                            agg = cnt32_sb[:, c0:c0 + cols]
                        else:
                            lhs1, lhs2 = w1_sb[:, 0:D], w1_sb[:, D:2 * D]
                            agg = aggr_b[:, 0:cols]
                        psh1 = ph.tile([P, MB * P], f32, tag="psh")
                        nc.tensor.matmul(out=psh1[:, 0:cols], lhsT=lhs1,
                                         rhs=agg, start=True, stop=True)
                        hidA = ap_.tile([P, MB * P], bf16, tag="hidA")
                        nc.scalar.activation(hidA[:, 0:cols], psh1[:, 0:cols],
                                             ACT.Relu, bias=b1a)
                        psh2 = ph.tile([P, MB * P], f32, tag="psh")
                        nc.tensor.matmul(out=psh2[:, 0:cols], lhsT=lhs2,
                                         rhs=agg, start=True, stop=True)
                        hidB = ap_.tile([P, MB * P], bf16, tag="hidB")
                        nc.scalar.activation(hidB[:, 0:cols], psh2[:, 0:cols],
                                             ACT.Relu, bias=b1b)

                        pso1 = po.tile([P, MB * P], f32, tag="pso")
                        nc.tensor.matmul(out=pso1[:, 0:cols], lhsT=w2a_sb[:],
                                         rhs=hidA[:, 0:cols], start=True, stop=False)
                        nc.tensor.matmul(out=pso1[:, 0:cols], lhsT=w2b_sb[:],
                                         rhs=hidB[:, 0:cols], start=False, stop=True)
                        outT_b = ap_.tile([P, MB * P], f32, tag="outT")
                        nc.vector.tensor_scalar_add(
                            outT_b[:, 0:cols], pso1[:, 0:cols], b2)

                        # ---- per-batch BN partial stats (hidden under gathers) ----
                        v1 = min(c0 + cols, NPC)
                        if v1 > c0:
                            nc.vector.tensor_reduce(
                                out=stats_p1[:, b:b + 1], in_=outT_b[:, 0:v1 - c0],
                                axis=mybir.AxisListType.X, op=ALU.add)
                            sq = ap_.tile([P, MB * P], bf16, tag="hidA")
                            nc.scalar.activation(
                                sq[:, 0:v1 - c0], outT_b[:, 0:v1 - c0], ACT.Square,
                                accum_out=stats_p2[:, b:b + 1])

                        # ---- early raw transposes to node-major ----
                        for nt in range(w0, w0 + w1n):
                            pst = pt.tile([P, P], f32, tag="pst")
                            nc.tensor.transpose(
                                out=pst[:],
                                in_=outT_b[:, (nt - w0) * P:(nt - w0 + 1) * P],
                                identity=idf_sb[:])
                            nc.scalar.activation(out_raw[:, nt * D:(nt + 1) * D],
                                                 pst[:], ACT.Copy)

                    # ---- batch-norm statistics ----
                    nc.scalar.activation(tmp_c[:], stats_p1[:, 0:1], ACT.Sqrt)
                    nc.vector.tensor_reduce(
                        out=stats_sb[:, 0:1], in_=stats_p1[:, 0:NBATCH],
                        axis=mybir.AxisListType.X, op=ALU.add)
                    nc.vector.tensor_reduce(
                        out=stats_sb[:, 1:2], in_=stats_p2[:, 0:NBATCH],
                        axis=mybir.AxisListType.X, op=ALU.add)
                    nc.sync.dma_start(out=st_loc[:, :], in_=stats_sb[:])
                    nc.gpsimd.collective_compute(
                        "AllReduce", ALU.add, replica_groups=RG,
                        ins=[st_loc.ap().opt()], outs=[st_glob.ap().opt()])
                    nc.sync.dma_start(out=gstats_sb[:], in_=st_glob[:, :])

                    # k = gamma*rstd (col0), c = beta - mean*k (col1)
                    nc.vector.tensor_scalar_mul(mean_c[:], gstats_sb[:, 0:1], inv_n)
                    nc.vector.tensor_scalar_mul(ex2_c[:], gstats_sb[:, 1:2], inv_n)
                    nc.scalar.activation(msq_c[:], mean_c[:], ACT.Square)
                    nc.vector.tensor_tensor(var_c[:], ex2_c[:], msq_c[:], op=ALU.subtract)
                    nc.vector.tensor_scalar_add(var_c[:], var_c[:], BN_EPS)
                    nc.scalar.activation(sd_c[:], var_c[:], ACT.Sqrt)
                    nc.vector.reciprocal(rstd_c[:], sd_c[:])
                    nc.vector.tensor_tensor(kc2[:, 0:1], gam, rstd_c[:], op=ALU.mult)
                    nc.vector.tensor_tensor(tmp_c[:], mean_c[:], kc2[:, 0:1], op=ALU.mult)
                    nc.vector.tensor_tensor(kc2[:, 1:2], bet, tmp_c[:], op=ALU.subtract)

                    # broadcast k,c across partitions: [128,1] -> row -> outer
                    pkc = pt.tile([P, P], f32, tag="pst")
                    nc.tensor.matmul(out=pkc[0:1, :], lhsT=kc2[:, 0:1], rhs=idf_sb[:],
                                     start=True, stop=True)
                    nc.scalar.activation(krow[:], pkc[0:1, :], ACT.Copy)
                    pcc = pt.tile([P, P], f32, tag="pst")
                    nc.tensor.matmul(out=pcc[0:1, :], lhsT=kc2[:, 1:2], rhs=idf_sb[:],
                                     start=True, stop=True)
                    nc.scalar.activation(crow[:], pcc[0:1, :], ACT.Copy)
                    pkb = pt.tile([P, P], f32, tag="pst")
                    nc.tensor.matmul(out=pkb[:], lhsT=ones1[:], rhs=krow[:],
                                     start=True, stop=True)
                    nc.scalar.activation(kb_sb[:], pkb[:], ACT.Copy)
                    pcb = pt.tile([P, P], f32, tag="pst")
                    nc.tensor.matmul(out=pcb[:], lhsT=ones1[:], rhs=crow[:],
                                     start=True, stop=True)
                    nc.scalar.activation(cb_sb[:], pcb[:], ACT.Copy)

                    # normalize in node-major: h = (relu?)(out_raw * kb + cb),
                    # chunked so scale/bias/relu/store pipeline across engines
                    def bcast(t, nw):
                        a = t[:]
                        return bass.AP(a.tensor, a.offset,
                                       [a.ap[0], [0, nw], a.ap[1]])


                    if l < L - 1:
                        # normalize in partition halves; each half's shard
                        # rows AllGather as soon as its store lands, so the
                        # A-stream gathers start after AG1 and AG2 hides
                        # under their descriptor generation
                        for qi, q0 in enumerate(range(0, NT, NCH)):
                            q1 = min(q0 + NCH, NT)
                            nw = q1 - q0
                            raw3 = out_raw[:, q0 * D:q1 * D].rearrange(
                                "p (t d) -> p t d", d=D)
                            nrm3 = nrm[:, q0 * D:q1 * D].rearrange(
                                "p (t d) -> p t d", d=D)
                            # all on DVE: engine-splitting these just halves
                            # both engines via SBUF contention
                            nc.vector.tensor_tensor(nrm3, raw3, bcast(kb_sb, nw),
                                                    op=ALU.mult)
                            nc.vector.tensor_tensor(raw3, nrm3, bcast(cb_sb, nw),
                                                    op=ALU.add)
                            nc.scalar.activation(h_nm[:, q0 * D:q1 * D],
                                                 out_raw[:, q0 * D:q1 * D],
                                                 ACT.Relu)
                            nc.sync.dma_start(
                                out=h_shard.ap().rearrange(
                                    "(p t) d -> p t d", t=NT)[:, q0:q1, :],
                                in_=h_nm[:, q0 * D:q1 * D].rearrange(
                                    "p (t d) -> p t d", d=D))
                        nc.gpsimd.collective_compute(
                            "AllGather", mybir.AluOpType.bypass, replica_groups=RG,
                            ins=[h_shard.ap().opt()], outs=[h_loc.ap().opt()])
                        # own-stream gathers for the next layer read h_shard
                        # (already final) -> their descriptor generation runs
                        # on the Q7 cores while the AllGathers are in flight
                        oc = {}
                        for ci in range((ET["ow"] + CH_TILES - 1) // CH_TILES):
                            ntile = min(CH_TILES, ET["ow"] - ci * CH_TILES)
                            nidx = ntile * P
                            g = owp.tile([P, CH_TILES * D], bf16, tag="own")
                            nc.gpsimd.dma_gather(
                                out_ap=g[:, 0:ntile * D].rearrange(
                                    "p (t e) -> p t e", e=D),
                                in_ap=h_shard[0:NPAD, :],
                                idxs_ap=gi["ow"][:, ci * CH_TILES * 8:
                                                 ci * CH_TILES * 8 + nidx // 16],
                                num_idxs=nidx, num_idxs_reg=nidx, elem_size=D,
                                single_packet=False, queue_num=0)
                            oc[ci] = g
                        own_chunks[l + 1] = oc
                    else:
                        # final output: f32, node-major -> [NPC, D]
                        for qi, q0 in enumerate(range(0, NT, NCH)):
                            q1 = min(q0 + NCH, NT)
                            nw = q1 - q0
                            raw3 = out_raw[:, q0 * D:q1 * D].rearrange(
                                "p (t d) -> p t d", d=D)
                            nrm3 = nrm[:, q0 * D:q1 * D].rearrange(
                                "p (t d) -> p t d", d=D)
                            nc.vector.tensor_tensor(nrm3, raw3, bcast(kb_sb, nw),
                                                    op=ALU.mult)
                            nc.vector.tensor_tensor(raw3, nrm3, bcast(cb_sb, nw),
                                                    op=ALU.add)
                            qf = min(q1, nfull)
                            if qf > q0:
                                nc.sync.dma_start(
                                    out=out_ext[q0 * P:qf * P, :].rearrange(
                                        "(t p) d -> p t d", p=P),
                                    in_=out_raw[:, q0 * D:qf * D].rearrange(
                                        "p (t d) -> p t d", d=D))
                            if q1 > nfull and rem:
                                nc.sync.dma_start(
                                    out=out_ext[nfull * P:NPC, :],
                                    in_=out_raw[0:rem, nfull * D:(nfull + 1) * D])

    # Align each gather's SWDGE queue with the DMASW semaphore lane Tile
    # assigned it (lane k <-> queue k % NQ), so no semaphore is shared by
    # two queues (completion order within a lane must match issue order).
    from concourse.tile_scheduler import PROC_NAME_TO_IDX
    dmasw0 = PROC_NAME_TO_IDX["DMASW0"]
    for inst in nc.inst_map.values():
        if isinstance(inst, mybir.InstDMAGatherAnt):
            proc = inst.bass_scheduled_proc
            assert proc is not None and dmasw0 <= proc < dmasw0 + 8, (
                f"gather {inst.name} not on a DMASW lane: {proc}")
            inst.queue_num = (proc - dmasw0) % NQ

    nc.compile()
    return nc


_CACHE = {}


def _sched_key(sched):
    return tuple((sched[s]["ET"], tuple(sched[s]["ta"]), tuple(sched[s]["tb"]))
                 for s in ("ow", "ev", "od"))


def _make_in_maps(arr, atom_emb0, atom_emb1, edge_emb0, edge_emb1,
                  W1, b1, W2, b2, gamma, beta):
    import ml_dtypes
    bf = ml_dtypes.bfloat16
    # ---- parameter tables (host float prep limited to tiny tables) ----
    ae0 = np.asarray(atom_emb0, np.float32)
    ae1 = np.asarray(atom_emb1, np.float32)
    ee0 = np.asarray(edge_emb0, np.float32)
    ee1 = np.asarray(edge_emb1, np.float32)
    t9 = np.zeros((16, D), np.float32)
    t9[:9] = (ae0[:3, None, :] + ae1[None, :3, :]).reshape(9, D)
    t15 = np.zeros((L, 16, D), np.float32)
    for l in range(L):
        t15[l, :15] = (ee0[l][:, None, :] + ee1[l][None, :, :]).reshape(15, D)

    W1 = np.asarray(W1, np.float32)
    W2 = np.asarray(W2, np.float32)
    b1 = np.asarray(b1, np.float32)
    b2 = np.asarray(b2, np.float32)
    gamma = np.asarray(gamma, np.float32)
    beta = np.asarray(beta, np.float32)
    prmT = np.zeros((P, 8 * L), np.float32)
    for l in range(L):
        prmT[:, l * 8 + 0] = b1[l, 0:D]
        prmT[:, l * 8 + 1] = b1[l, D:2 * D]
        prmT[:, l * 8 + 2] = b2[l]
        prmT[:, l * 8 + 3] = gamma[l]
        prmT[:, l * 8 + 4] = beta[l]

    # layer-0 effective first-layer weight: [t9; t15_0] @ W1[0]
    T32 = np.concatenate([t9, t15[0]], axis=0)          # [32, D]
    w1e = T32.astype(bf).astype(np.float32) @ W1[0].astype(bf).astype(np.float32)

    ident = np.eye(P, dtype=np.float32)

    in_maps = []
    for c in range(NCORES):
        m = {
            "cnt": arr["cnt"][c].astype(bf),
            "cnt32": arr["cnt32"][c].astype(bf),
            "w1e": w1e.astype(bf),
            "prmT": prmT,
            "w1": W1.astype(bf),
            "w2": W2.astype(bf),
            "t15": t15.astype(bf),
            "identf": ident,
            "identb": ident.astype(ml_dtypes.float8_e4m3),
        }
        for s in ("ow", "ev", "od"):
            m[f"gidx_{s}"] = _wrap_idx_cols(arr[f"gidx_{s}"][c])
            oh = arr[f"oh_{s}"][c]            # [TOT, P, P] uint8
            m[f"oh_{s}"] = np.ascontiguousarray(
                oh.transpose(1, 0, 2)).reshape(P, -1).astype(
                    ml_dtypes.float8_e4m3)
        in_maps.append(m)
    return in_maps


def kernel(x, edge_index, edge_attr, atom_emb0, atom_emb1,
           edge_emb0, edge_emb1, W1, b1, W2, b2, gamma, beta):
    from concourse.bass_utils import run_bass_kernel_spmd

    sched, arr = _preprocess(x, edge_index, edge_attr)
    key = _sched_key(sched)
    if key not in _CACHE:
        _CACHE[key] = _build(sched)
    nc = _CACHE[key]

    in_maps = _make_in_maps(arr, atom_emb0, atom_emb1, edge_emb0, edge_emb1,
                            W1, b1, W2, b2, gamma, beta)
    res = run_bass_kernel_spmd(nc, in_maps, core_ids=list(range(NCORES)))
    out = np.concatenate([res.results[c]["out"] for c in range(NCORES)], axis=0)
    return out.astype(np.float32)
